# revision 31
# baseline (speedup 1.0000x reference)
"""Trainium2 Bass kernel for y = 2*(einsum('bct,oc->bot', pre, W_pre) + b_pre).

Shapes (hardcoded): pre [16, 512, 4096] f32, W_pre [512, 512] f32, b_pre [512] f32.
Sharding: data-parallel over B across 8 cores (2 batches per core).

Per core: out[b, o, t] = 2*(sum_c W[o,c]*pre[b,c,t] + bias[o]) for 2 batches.
PE matmul computes lhsT.T @ rhs with lhsT = W.T tiles [K=128, M=128] and
rhs = pre tiles [K=128, N<=512]; accumulate 4 K-tiles into one PSUM bank,
then ScalarE/DVE apply out = psum + 2*bias on eviction PSUM->SBUF.

All device I/O is fp16 (host casts): the fp32 baseline was HBM-bound
(33.6MB/core at ~358GB/s ~ 94us); fp16 halves traffic to 16.8MB (~47us),
making the kernel PE-bound (~55us of fp16-rate matmul). fp8 would double
the PE rate (DoubleRow) but exceeds the 2e-2 error gate (4e-2 measured
offline on the exact dataset), so fp16 is the precision floor.

Startup critical path: HWDGE issue costs ~0.6us+ per DMA on the issuing
sequencer and the engines deliver ~250GB/s early on, so the first-matmul
wait is set by (first x chunk + first W half). Both are 256KB: x loads
issue on SP (sync), W (mt-major, 2 half DMAs) + bias on ACT (scalar).
Warmup matmuls on a memset tile run during the DMA wait so the PE HAM
clock gate is already released (2.4GHz) when real matmuls start.
Outputs stream from SBUF on both HWDGE rings, tapered so the last DMAs
after the final matmul are small.
"""

import os
import sys

for _p in ("/opt/trn_rl_repo", "/root/.axon_site/_ro/trn_rl_repo"):
    if os.path.isdir(_p) and _p not in sys.path:
        sys.path.append(_p)

from contextlib import ExitStack

import numpy as np

import concourse.bass as bass
import concourse.tile as tile
from concourse import bacc, mybir
from concourse.bass_utils import run_bass_kernel_spmd

B, C, T = 16, 512, 4096  # batch, channels (in == out), sequence
NCORES = 8
BPC = B // NCORES  # batches per core
P = 128
KT = C // P  # contraction tiles
MT = C // P  # output-channel tiles
NCHUNK = 512  # max matmul moving-operand free dim (PSUM bank limit)
# Input DMA column chunks: small first chunks so the first matmul group's
# data lands early, bigger later ones to amortize DMA issue overhead.
XCS = [256, 256, 512, 1024, 2048]
# Matmul work chunks (cols per PSUM group), derived from XCS boundaries.
WORK = []  # (xi, xoff, ncols)
for _xi, _c in enumerate(XCS):
    for _o in range(0, _c, NCHUNK):
        WORK.append((_xi, _o, min(NCHUNK, _c - _o)))
NCHW = len(WORK)
# Per-batch work order and output store groups (lists of WORK indices per
# otile; each group must cover contiguous columns). Batch 1 processes the
# two leading 256-col chunks LAST so the final evictions + out DMAs after
# the last matmul are as small as possible.
OGS = {
    0: [list(range(NCHW))],
    1: [[5, 6, 7, 8], [3, 4], [2], [1], [0]],
}

IN_DT = mybir.dt.float16
OUT_DT = mybir.dt.float16

LAST_RESULT = None  # BassKernelResults of the most recent run (for test harness)
_cache = {}


def _build():
    # Bacc (not plain Bass): its finalize() runs move_matmul_waits_to_ldweights +
    # generate_event_semaphores, which walrus needs.
    nc = bacc.Bacc("TRN2", target_bir_lowering=False, debug=False, num_devices=NCORES)
    # pre viewed as [b, kt, p, t] (same layout as [b, c, t] with c = kt*128+p).
    pre = nc.dram_tensor("pre", [BPC, KT, P, T], IN_DT, kind="ExternalInput").ap()
    # W pre-tiled on host, mt-major: wt[p, mt*KT+kt, m] = 2*W[mt*128+m, kt*128+p]
    # so each half-DMA delivers complete mt groups.
    wt = nc.dram_tensor("wt", [P, KT * MT, P], IN_DT, kind="ExternalInput").ap()
    b2 = nc.dram_tensor("b2", [P, MT], mybir.dt.float32, kind="ExternalInput").ap()
    out = nc.dram_tensor("out", [BPC, C, T], OUT_DT, kind="ExternalOutput").ap()

    with ExitStack() as ctx:
        tc = ctx.enter_context(tile.TileContext(nc))
        wpool = ctx.enter_context(tc.tile_pool(name="w", bufs=1))
        bpool = ctx.enter_context(tc.tile_pool(name="bias", bufs=1))
        xpool = ctx.enter_context(tc.tile_pool(name="x", bufs=2))
        opool = ctx.enter_context(tc.tile_pool(name="o", bufs=8))
        pspool = ctx.enter_context(tc.tile_pool(name="ps", bufs=8, space="PSUM"))

        # One DMA per (batch, column chunk), covering all 4 K-tiles: SBUF
        # tile [128, KT, cols] <- dram [kt, p, cols] transposed to [p, kt, cols].
        # Issue order is consumption order; b0 chunk 0 first.
        def load_x(b, xi, off, cols):
            x = xpool.tile([P, KT, cols], IN_DT, name=f"x_{b}_{xi}", tag=f"x{xi}")
            nc.sync.dma_start(
                x[:], pre[b, :, :, bass.ds(off, cols)].transpose([1, 0, 2])
            )
            return x

        xtiles = {}
        xtiles[(0, 0)] = load_x(0, 0, 0, XCS[0])

        # W in four quarter DMAs on the other HWDGE engine (ACT), mt-major so
        # each 128KB quarter delivers one complete mt group in consumption
        # order; the critical mt0 piece lands ~1us earlier than a half would.
        # Bias rides the gpsimd (SWDGE) ring so it never queues behind W.
        wtile = wpool.tile([P, KT * MT * P], IN_DT, name="w")
        Q = KT * P
        for mt in range(MT):
            nc.scalar.dma_start(
                wtile[:, mt * Q : (mt + 1) * Q], wt[:, mt * KT : (mt + 1) * KT, :]
            )
        btile = bpool.tile([P, MT], mybir.dt.float32)
        nc.gpsimd.dma_start(btile[:], b2[:])

        def wslice(kt, mt):
            return wtile[:, (mt * KT + kt) * P : (mt * KT + kt + 1) * P]

        # Warmup matmuls on a memset tile while the first x/W DMAs are in
        # flight: ~4us of sustained PE activity releases the HAM clock gate
        # (1.2 -> 2.4 GHz) before the first real matmul arrives. Small-N
        # matmuls keep the PE busy right up to the moment the real data
        # lands while delaying the first real matmul by at most ~0.1us.
        warm = bpool.tile([P, NCHUNK], IN_DT, name="warm")
        nc.vector.memset(warm[:], 0)
        ps_warm = pspool.tile([P, NCHUNK], mybir.dt.float32, tag="ps")
        for _ in range(36):
            nc.tensor.matmul(
                ps_warm[:, 0:P], warm[:, 0:P], warm[:, 0:P], start=True, stop=True
            )

        off = XCS[0]
        for xi in range(1, len(XCS)):
            xtiles[(0, xi)] = load_x(0, xi, off, XCS[xi])
            off += XCS[xi]
        off = 0
        for xi in range(len(XCS)):
            xtiles[(1, xi)] = load_x(1, xi, off, XCS[xi])
            off += XCS[xi]

        colbase = [sum(c[2] for c in WORK[:i]) for i in range(NCHW)]
        evict = 0
        for b in range(BPC):
            for og, group in enumerate(OGS[b]):
                chunks = [WORK[i] for i in group]
                ocols = sum(c[2] for c in chunks)
                obase = min(colbase[i] for i in group)
                otiles = [
                    opool.tile([P, ocols], OUT_DT, name=f"o_{b}_{og}_{mt}", tag="o")
                    for mt in range(MT)
                ]
                for i in group:
                    xi, xoff, ncols = WORK[i]
                    ooff = colbase[i] - obase
                    for mt in range(MT):
                        ps = pspool.tile([P, ncols], mybir.dt.float32, tag="ps")
                        for kt in range(KT):
                            nc.tensor.matmul(
                                ps[:],
                                wslice(kt, mt),
                                xtiles[(b, xi)][:, kt, xoff : xoff + ncols],
                                start=(kt == 0),
                                stop=(kt == KT - 1),
                            )
                        # W is pre-scaled by 2 on the host, so only + 2*bias
                        # remains; alternate DVE/ACT so neither engine binds.
                        dst = otiles[mt][:, ooff : ooff + ncols]
                        bias_col = btile[:, mt : mt + 1]
                        evict += 1
                        if evict % 2 == 0:
                            nc.vector.tensor_scalar_add(dst, ps[:], bias_col)
                        else:
                            nc.scalar.activation(
                                dst,
                                ps[:],
                                mybir.ActivationFunctionType.Identity,
                                bias=bias_col,
                            )
                for mt in range(MT):
                    # Alternate output DMAs across both HWDGE rings so the
                    # tail's issue cost (~0.6us each) is split.
                    eng = nc.scalar if mt % 2 else nc.sync
                    eng.dma_start(
                        out[b, mt * P : (mt + 1) * P, bass.ds(obase, ocols)],
                        otiles[mt][:],
                    )
    # The axon/PJRT exec path serializes nc as-is; finalize here so Bacc's
    # compile passes (register alloc, event-semaphore wait splitting) run.
    nc.finalize()
    return nc


def kernel(pre, W_pre, b_pre):
    global LAST_RESULT
    preh = np.ascontiguousarray(np.asarray(pre, dtype=np.float32).astype(np.float16))
    # Fold the reference's final y+y into the weights/bias: out = (2W)x + 2b.
    # Pre-tile W mt-major: wt[p, mt*KT+kt, m] = 2*W[mt*128+m, kt*128+p].
    w2 = (np.asarray(W_pre, dtype=np.float32) * 2.0).astype(np.float16)
    wtil = np.ascontiguousarray(
        w2.reshape(MT, P, KT, P).transpose(3, 0, 2, 1).reshape(P, KT * MT, P)
    )  # [p, mt, kt, m]
    b2 = np.ascontiguousarray(
        (2.0 * np.asarray(b_pre, dtype=np.float32)).reshape(MT, P).T
    )
    if "nc" not in _cache:
        _cache["nc"] = _build()
    nc = _cache["nc"]
    in_maps = [
        {"pre": preh[i * BPC : (i + 1) * BPC], "wt": wtil, "b2": b2}
        for i in range(NCORES)
    ]
    res = run_bass_kernel_spmd(nc, in_maps, list(range(NCORES)))
    LAST_RESULT = res
    return np.ascontiguousarray(
        np.concatenate([res.results[i]["out"] for i in range(NCORES)], axis=0),
        dtype=np.float32,
    )


# revision 32
# speedup vs baseline: 1.0145x; 1.0145x over previous
"""Trainium2 Bass kernel for y = 2*(einsum('bct,oc->bot', pre, W_pre) + b_pre).

Shapes (hardcoded): pre [16, 512, 4096] f32, W_pre [512, 512] f32, b_pre [512] f32.
Sharding: data-parallel over B across 8 cores (2 batches per core).

Per core: out[b, o, t] = 2*(sum_c W[o,c]*pre[b,c,t] + bias[o]) for 2 batches.
PE matmul computes lhsT.T @ rhs with lhsT = W.T tiles [K=128, M=128] and
rhs = pre tiles [K=128, N<=512]; accumulate 4 K-tiles into one PSUM bank,
then ScalarE/DVE apply out = psum + 2*bias on eviction PSUM->SBUF.

All device I/O is fp16 (host casts): the fp32 baseline was HBM-bound
(33.6MB/core at ~358GB/s ~ 94us); fp16 halves traffic to 16.8MB (~47us),
making the kernel PE-bound (~55us of fp16-rate matmul). fp8 would double
the PE rate (DoubleRow) but exceeds the 2e-2 error gate (4e-2 measured
offline on the exact dataset), so fp16 is the precision floor.

Startup critical path: HWDGE issue costs ~0.6us+ per DMA on the issuing
sequencer and the engines deliver ~250GB/s early on, so the first-matmul
wait is set by (first x chunk + first W half). Both are 256KB: x loads
issue on SP (sync), W (mt-major, 2 half DMAs) + bias on ACT (scalar).
Warmup matmuls on a memset tile run during the DMA wait so the PE HAM
clock gate is already released (2.4GHz) when real matmuls start.
Outputs stream from SBUF on both HWDGE rings, tapered so the last DMAs
after the final matmul are small.
"""

import os
import sys

for _p in ("/opt/trn_rl_repo", "/root/.axon_site/_ro/trn_rl_repo"):
    if os.path.isdir(_p) and _p not in sys.path:
        sys.path.append(_p)

from contextlib import ExitStack

import numpy as np

import concourse.bass as bass
import concourse.tile as tile
from concourse import bacc, mybir
from concourse.bass_utils import run_bass_kernel_spmd

B, C, T = 16, 512, 4096  # batch, channels (in == out), sequence
NCORES = 8
BPC = B // NCORES  # batches per core
P = 128
KT = C // P  # contraction tiles
MT = C // P  # output-channel tiles
NCHUNK = 512  # max matmul moving-operand free dim (PSUM bank limit)
# Input DMA column chunks: small first chunks so the first matmul group's
# data lands early, bigger later ones to amortize DMA issue overhead.
XCS = [256, 256, 512, 1024, 2048]
# Matmul work chunks (cols per PSUM group), derived from XCS boundaries.
WORK = []  # (xi, xoff, ncols)
for _xi, _c in enumerate(XCS):
    for _o in range(0, _c, NCHUNK):
        WORK.append((_xi, _o, min(NCHUNK, _c - _o)))
NCHW = len(WORK)
# Per-batch work order and output store groups (lists of WORK indices per
# otile; each group must cover contiguous columns). Batch 1 processes the
# two leading 256-col chunks LAST so the final evictions + out DMAs after
# the last matmul are as small as possible.
OGS = {
    0: [list(range(NCHW))],
    1: [[5, 6, 7, 8], [3, 4], [2], [1], [0]],
}

IN_DT = mybir.dt.float16
OUT_DT = mybir.dt.float16

LAST_RESULT = None  # BassKernelResults of the most recent run (for test harness)
_cache = {}


def _build():
    # Bacc (not plain Bass): its finalize() runs move_matmul_waits_to_ldweights +
    # generate_event_semaphores, which walrus needs.
    nc = bacc.Bacc("TRN2", target_bir_lowering=False, debug=False, num_devices=NCORES)
    # pre viewed as [b, kt, p, t] (same layout as [b, c, t] with c = kt*128+p).
    pre = nc.dram_tensor("pre", [BPC, KT, P, T], IN_DT, kind="ExternalInput").ap()
    # W pre-tiled on host, mt-major: wt[p, mt*KT+kt, m] = 2*W[mt*128+m, kt*128+p]
    # so each half-DMA delivers complete mt groups.
    wt = nc.dram_tensor("wt", [P, KT * MT, P], IN_DT, kind="ExternalInput").ap()
    b2 = nc.dram_tensor("b2", [P, MT], mybir.dt.float32, kind="ExternalInput").ap()
    out = nc.dram_tensor("out", [BPC, C, T], OUT_DT, kind="ExternalOutput").ap()

    with ExitStack() as ctx:
        tc = ctx.enter_context(tile.TileContext(nc))
        wpool = ctx.enter_context(tc.tile_pool(name="w", bufs=1))
        bpool = ctx.enter_context(tc.tile_pool(name="bias", bufs=1))
        xpool = ctx.enter_context(tc.tile_pool(name="x", bufs=2))
        opool = ctx.enter_context(tc.tile_pool(name="o", bufs=8))
        pspool = ctx.enter_context(tc.tile_pool(name="ps", bufs=8, space="PSUM"))

        # One DMA per (batch, column chunk), covering all 4 K-tiles: SBUF
        # tile [128, KT, cols] <- dram [kt, p, cols] transposed to [p, kt, cols].
        # Issue order is consumption order; b0 chunk 0 first.
        def load_x(b, xi, off, cols):
            x = xpool.tile([P, KT, cols], IN_DT, name=f"x_{b}_{xi}", tag=f"x{xi}")
            nc.sync.dma_start(
                x[:], pre[b, :, :, bass.ds(off, cols)].transpose([1, 0, 2])
            )
            return x

        xtiles = {}
        xtiles[(0, 0)] = load_x(0, 0, 0, XCS[0])

        # W in two half DMAs on the other HWDGE engine (ACT), mt-major so the
        # first 256KB covers mt 0-1 completely (the first PSUM groups).
        wtile = wpool.tile([P, KT * MT * P], IN_DT, name="w")
        HW = KT * MT * P // 2
        nc.scalar.dma_start(wtile[:, 0:HW], wt[:, 0 : KT * MT // 2, :])
        nc.scalar.dma_start(wtile[:, HW:], wt[:, KT * MT // 2 :, :])
        btile = bpool.tile([P, MT], mybir.dt.float32)
        nc.scalar.dma_start(btile[:], b2[:])

        def wslice(kt, mt):
            return wtile[:, (mt * KT + kt) * P : (mt * KT + kt + 1) * P]

        # Warmup matmuls on a memset tile while the first x/W DMAs are in
        # flight: ~4us of sustained PE activity releases the HAM clock gate
        # (1.2 -> 2.4 GHz) before the first real matmul arrives. Small-N
        # matmuls keep the PE busy right up to the moment the real data
        # lands while delaying the first real matmul by at most ~0.1us.
        warm = bpool.tile([P, NCHUNK], IN_DT, name="warm")
        nc.vector.memset(warm[:], 0)
        ps_warm = pspool.tile([P, NCHUNK], mybir.dt.float32, tag="ps")
        for _ in range(38):
            nc.tensor.matmul(
                ps_warm[:, 0:P], warm[:, 0:P], warm[:, 0:P], start=True, stop=True
            )

        off = XCS[0]
        for xi in range(1, len(XCS)):
            xtiles[(0, xi)] = load_x(0, xi, off, XCS[xi])
            off += XCS[xi]
        off = 0
        for xi in range(len(XCS)):
            xtiles[(1, xi)] = load_x(1, xi, off, XCS[xi])
            off += XCS[xi]

        colbase = [sum(c[2] for c in WORK[:i]) for i in range(NCHW)]
        evict = 0
        for b in range(BPC):
            for og, group in enumerate(OGS[b]):
                chunks = [WORK[i] for i in group]
                ocols = sum(c[2] for c in chunks)
                obase = min(colbase[i] for i in group)
                otiles = [
                    opool.tile([P, ocols], OUT_DT, name=f"o_{b}_{og}_{mt}", tag="o")
                    for mt in range(MT)
                ]
                for i in group:
                    xi, xoff, ncols = WORK[i]
                    ooff = colbase[i] - obase
                    for mt in range(MT):
                        ps = pspool.tile([P, ncols], mybir.dt.float32, tag="ps")
                        for kt in range(KT):
                            nc.tensor.matmul(
                                ps[:],
                                wslice(kt, mt),
                                xtiles[(b, xi)][:, kt, xoff : xoff + ncols],
                                start=(kt == 0),
                                stop=(kt == KT - 1),
                            )
                        # W is pre-scaled by 2 on the host, so only + 2*bias
                        # remains; alternate DVE/ACT so neither engine binds.
                        dst = otiles[mt][:, ooff : ooff + ncols]
                        bias_col = btile[:, mt : mt + 1]
                        evict += 1
                        if evict % 2 == 0:
                            nc.vector.tensor_scalar_add(dst, ps[:], bias_col)
                        else:
                            nc.scalar.activation(
                                dst,
                                ps[:],
                                mybir.ActivationFunctionType.Identity,
                                bias=bias_col,
                            )
                for mt in range(MT):
                    # Alternate output DMAs across both HWDGE rings so the
                    # tail's issue cost (~0.6us each) is split.
                    eng = nc.scalar if mt % 2 else nc.sync
                    eng.dma_start(
                        out[b, mt * P : (mt + 1) * P, bass.ds(obase, ocols)],
                        otiles[mt][:],
                    )
    # The axon/PJRT exec path serializes nc as-is; finalize here so Bacc's
    # compile passes (register alloc, event-semaphore wait splitting) run.
    nc.finalize()
    return nc


def kernel(pre, W_pre, b_pre):
    global LAST_RESULT
    preh = np.ascontiguousarray(np.asarray(pre, dtype=np.float32).astype(np.float16))
    # Fold the reference's final y+y into the weights/bias: out = (2W)x + 2b.
    # Pre-tile W mt-major: wt[p, mt*KT+kt, m] = 2*W[mt*128+m, kt*128+p].
    w2 = (np.asarray(W_pre, dtype=np.float32) * 2.0).astype(np.float16)
    wtil = np.ascontiguousarray(
        w2.reshape(MT, P, KT, P).transpose(3, 0, 2, 1).reshape(P, KT * MT, P)
    )  # [p, mt, kt, m]
    b2 = np.ascontiguousarray(
        (2.0 * np.asarray(b_pre, dtype=np.float32)).reshape(MT, P).T
    )
    if "nc" not in _cache:
        _cache["nc"] = _build()
    nc = _cache["nc"]
    in_maps = [
        {"pre": preh[i * BPC : (i + 1) * BPC], "wt": wtil, "b2": b2}
        for i in range(NCORES)
    ]
    res = run_bass_kernel_spmd(nc, in_maps, list(range(NCORES)))
    LAST_RESULT = res
    return np.ascontiguousarray(
        np.concatenate([res.results[i]["out"] for i in range(NCORES)], axis=0),
        dtype=np.float32,
    )
